# revision 7
# baseline (speedup 1.0000x reference)
"""LoRALinear TRN2 kernel: y = x @ W^T + bias + 2.0 * (x @ A^T) @ B^T.

Strategy
--------
- Host-side fold: Wf = W + 2.0 * (B @ A)  (LoRA merge — algebraically exact),
  so the device kernel is a plain tiled GEMM + bias.
- Data-parallel over tokens: 8 cores x 1024 tokens, Wf/bias replicated.
- Per core we compute y_c^T [4096 out, 1024 tok] so the PSUM->SBUF eviction
  can fuse the bias as a per-partition scalar add on VectorE.
- matmul(out, lhsT, rhs) = lhsT.T @ rhs: lhsT = Wf^T tile [k=128, o=128]
  (stationary), rhs = x^T tile [k=128, t=512] (moving), accumulate over 32
  k-tiles into PSUM. 8 PSUM banks hold the (4 o_sub x 2 t_blk) accumulators
  of one 512-wide output chunk while W streams through a small SBUF pool.
- x^T stays fully resident in SBUF; W streams once (W-stationary-in-PE,
  x-resident-in-SBUF, out written once).
"""

import sys

import numpy as np

if "/opt/trn_rl_repo" not in sys.path:
    sys.path.insert(0, "/opt/trn_rl_repo")

from concourse import bacc, bass, mybir, tile
from concourse.bass_utils import run_bass_kernel_spmd

TOKENS, D_IN, D_OUT, R = 8192, 4096, 4096, 16
SCALE = 2.0
N_CORES = 8
T_C = TOKENS // N_CORES  # 1024 tokens per core
KT = D_IN // 128  # 32 contraction tiles
OC = D_OUT // 512  # 8 output chunks of 512
OSUB = 4  # 128-wide o blocks per chunk
TB = T_C // 512  # 2 moving-dim blocks of 512

# float32r: fp32 storage, reduced-precision multiply at bf16 rate (N>=256).
COMPUTE_DT = mybir.dt.float32r

_NC_CACHE: dict = {}


def _build(compute_dt):
    nc = bacc.Bacc("TRN2", target_bir_lowering=False, debug=False)
    f32 = mybir.dt.float32

    xt_d = nc.dram_tensor("xt", [128, KT, T_C], compute_dt, kind="ExternalInput")
    wt_d = nc.dram_tensor("wt", [OC, KT, 128, 512], compute_dt, kind="ExternalInput")
    b_d = nc.dram_tensor("bias", [128, OC * OSUB], f32, kind="ExternalInput")
    out_d = nc.dram_tensor("out", [D_OUT, T_C], f32, kind="ExternalOutput")

    with tile.TileContext(nc) as tc:
        with (
            tc.tile_pool(name="xp", bufs=1) as xp,
            tc.tile_pool(name="wp", bufs=20) as wp,
            tc.tile_pool(name="pp", bufs=8, space=bass.MemorySpace.PSUM) as pp,
            tc.tile_pool(name="op", bufs=6) as op,
            tc.tile_pool(name="bp", bufs=1) as bp,
        ):
            bb = bp.tile([128, OC * OSUB], f32)
            nc.gpsimd.dma_start(bb[:], b_d[:])
            xbuf = xp.tile([128, KT, T_C], compute_dt)

            for oc in range(OC):
                pss = {}
                for os_ in range(OSUB):
                    for tb in range(TB):
                        pss[os_, tb] = pp.tile(
                            [128, 512], f32, tag="ps", name=f"ps_{oc}_{os_}_{tb}"
                        )
                for kt in range(KT):
                    if oc == 0:
                        # lazy x load, in consumption order, own SWDGE queue
                        nc.gpsimd.dma_start(xbuf[:, kt, :], xt_d[:, kt, :])
                    w = wp.tile([128, 512], compute_dt, tag="w")
                    nc.sync.dma_start(w[:], wt_d[oc, kt])
                    for os_ in range(OSUB):
                        lhs = w[:, os_ * 128 : (os_ + 1) * 128]
                        for tb in range(TB):
                            nc.tensor.matmul(
                                pss[os_, tb][:],
                                lhs,
                                xbuf[:, kt, tb * 512 : (tb + 1) * 512],
                                start=(kt == 0),
                                stop=(kt == KT - 1),
                            )
                for os_ in range(OSUB):
                    ob = oc * OSUB + os_
                    for tb in range(TB):
                        ot = op.tile([128, 512], f32, tag="o")
                        nc.vector.tensor_scalar_add(
                            ot[:], pss[os_, tb][:], bb[:, ob : ob + 1]
                        )
                        nc.scalar.dma_start(
                            out_d[ob * 128 : (ob + 1) * 128, tb * 512 : (tb + 1) * 512],
                            ot[:],
                        )
    nc.compile()
    return nc


def _get_nc(compute_dt):
    if compute_dt not in _NC_CACHE:
        _NC_CACHE[compute_dt] = _build(compute_dt)
    return _NC_CACHE[compute_dt]


def kernel(x, weight, bias, lora_A, lora_B, _trace=False, _compute_dt=None, **_kw):
    compute_dt = COMPUTE_DT if _compute_dt is None else _compute_dt
    np_cdt = np.dtype(mybir.dt.np(compute_dt))

    x = np.asarray(x, np.float32)
    weight = np.asarray(weight, np.float32)
    bias = np.asarray(bias, np.float32)
    lora_A = np.asarray(lora_A, np.float32)
    lora_B = np.asarray(lora_B, np.float32)

    wf = weight + np.float32(SCALE) * (lora_B @ lora_A)  # [O, I]
    # wt_prep[oc, kt, kp, oo] = wf[oc*512+oo, kt*128+kp]
    wt_prep = np.ascontiguousarray(
        wf.T.reshape(KT, 128, OC, 512).transpose(2, 0, 1, 3)
    ).astype(np_cdt)
    bias_prep = np.ascontiguousarray(bias.reshape(OC * OSUB, 128).T)  # [128, 32]

    in_maps = []
    for c in range(N_CORES):
        xs = x[c * T_C : (c + 1) * T_C]  # [1024, 4096]
        # xt[kp, kt, t] = xs[t, kt*128+kp]
        xt = np.ascontiguousarray(
            xs.T.reshape(KT, 128, T_C).transpose(1, 0, 2)
        ).astype(np_cdt)
        in_maps.append({"xt": xt, "wt": wt_prep, "bias": bias_prep})

    nc = _get_nc(compute_dt)
    res = run_bass_kernel_spmd(nc, in_maps, list(range(N_CORES)), trace=_trace)
    kernel.last_results = res

    y = np.empty((TOKENS, D_OUT), np.float32)
    for c in range(N_CORES):
        y[c * T_C : (c + 1) * T_C] = res.results[c]["out"].T
    return y


# revision 9
# speedup vs baseline: 1.1017x; 1.1017x over previous
"""LoRALinear TRN2 kernel: y = x @ W^T + bias + 2.0 * (x @ A^T) @ B^T.

Strategy
--------
- Host-side fold: Wf = W + 2.0 * (B @ A)  (LoRA merge — algebraically exact),
  so the device kernel is a plain tiled GEMM + bias.
- Data-parallel over tokens: 8 cores x 1024 tokens, Wf/bias replicated.
- Per core we compute y_c^T [4096 out, 1024 tok] so the PSUM->SBUF eviction
  can fuse the bias as a per-partition scalar add on VectorE.
- matmul(out, lhsT, rhs) = lhsT.T @ rhs: lhsT = Wf^T tile [k=128, o=128]
  (stationary), rhs = x^T tile [k=128, t=512] (moving), accumulate over 32
  k-tiles into PSUM. 8 PSUM banks hold the (4 o_sub x 2 t_blk) accumulators
  of one 512-wide output chunk while W streams through a small SBUF pool.
- x^T stays fully resident in SBUF; W streams once (W-stationary-in-PE,
  x-resident-in-SBUF, out written once).
"""

import sys

import numpy as np

if "/opt/trn_rl_repo" not in sys.path:
    sys.path.insert(0, "/opt/trn_rl_repo")

from concourse import bacc, bass, mybir, tile
from concourse.bass_utils import run_bass_kernel_spmd

TOKENS, D_IN, D_OUT, R = 8192, 4096, 4096, 16
SCALE = 2.0
N_CORES = 8
T_C = TOKENS // N_CORES  # 1024 tokens per core
KT = D_IN // 128  # 32 contraction tiles
OC = D_OUT // 512  # 8 output chunks of 512
OSUB = 4  # 128-wide o blocks per chunk
TB = T_C // 512  # 2 moving-dim blocks of 512

# float32r: fp32 storage, reduced-precision multiply at bf16 rate (N>=256).
COMPUTE_DT = mybir.dt.float32r

_NC_CACHE: dict = {}


def _build(compute_dt):
    nc = bacc.Bacc("TRN2", target_bir_lowering=False, debug=False)
    f32 = mybir.dt.float32

    xt_d = nc.dram_tensor("xt", [128, KT, T_C], compute_dt, kind="ExternalInput")
    wt_d = nc.dram_tensor("wt", [OC, KT, 128, 512], compute_dt, kind="ExternalInput")
    b_d = nc.dram_tensor("bias", [128, OC * OSUB], f32, kind="ExternalInput")
    out_d = nc.dram_tensor("out", [D_OUT, T_C], f32, kind="ExternalOutput")

    with tile.TileContext(nc) as tc:
        with (
            tc.tile_pool(name="xp", bufs=1) as xp,
            tc.tile_pool(name="wp", bufs=20) as wp,
            tc.tile_pool(name="pp", bufs=8, space=bass.MemorySpace.PSUM) as pp,
            tc.tile_pool(name="op", bufs=6) as op,
            tc.tile_pool(name="bp", bufs=1) as bp,
        ):
            bb = bp.tile([128, OC * OSUB], f32)
            nc.sync.dma_start(bb[:], b_d[:])
            xbuf = xp.tile([128, KT, T_C], compute_dt)

            for oc in range(OC):
                pss = {}
                for os_ in range(OSUB):
                    for tb in range(TB):
                        pss[os_, tb] = pp.tile(
                            [128, 512], f32, tag="ps", name=f"ps_{oc}_{os_}_{tb}"
                        )
                for kt in range(KT):
                    if oc == 0:
                        # lazy x load in consumption order, on the scalar HWDGE
                        # queue which is otherwise idle until stores begin
                        nc.scalar.dma_start(xbuf[:, kt, :], xt_d[:, kt, :])
                    w = wp.tile([128, 512], compute_dt, tag="w")
                    nc.sync.dma_start(w[:], wt_d[oc, kt])
                    for os_ in range(OSUB):
                        lhs = w[:, os_ * 128 : (os_ + 1) * 128]
                        for tb in range(TB):
                            nc.tensor.matmul(
                                pss[os_, tb][:],
                                lhs,
                                xbuf[:, kt, tb * 512 : (tb + 1) * 512],
                                start=(kt == 0),
                                stop=(kt == KT - 1),
                            )
                for os_ in range(OSUB):
                    ob = oc * OSUB + os_
                    for tb in range(TB):
                        ot = op.tile([128, 512], f32, tag="o")
                        nc.vector.tensor_scalar_add(
                            ot[:], pss[os_, tb][:], bb[:, ob : ob + 1]
                        )
                        nc.scalar.dma_start(
                            out_d[ob * 128 : (ob + 1) * 128, tb * 512 : (tb + 1) * 512],
                            ot[:],
                        )
    nc.compile()
    return nc


def _get_nc(compute_dt):
    if compute_dt not in _NC_CACHE:
        _NC_CACHE[compute_dt] = _build(compute_dt)
    return _NC_CACHE[compute_dt]


def kernel(x, weight, bias, lora_A, lora_B, _trace=False, _compute_dt=None, **_kw):
    compute_dt = COMPUTE_DT if _compute_dt is None else _compute_dt
    np_cdt = np.dtype(mybir.dt.np(compute_dt))

    x = np.asarray(x, np.float32)
    weight = np.asarray(weight, np.float32)
    bias = np.asarray(bias, np.float32)
    lora_A = np.asarray(lora_A, np.float32)
    lora_B = np.asarray(lora_B, np.float32)

    wf = weight + np.float32(SCALE) * (lora_B @ lora_A)  # [O, I]
    # wt_prep[oc, kt, kp, oo] = wf[oc*512+oo, kt*128+kp]
    wt_prep = np.ascontiguousarray(
        wf.T.reshape(KT, 128, OC, 512).transpose(2, 0, 1, 3)
    ).astype(np_cdt)
    bias_prep = np.ascontiguousarray(bias.reshape(OC * OSUB, 128).T)  # [128, 32]

    in_maps = []
    for c in range(N_CORES):
        xs = x[c * T_C : (c + 1) * T_C]  # [1024, 4096]
        # xt[kp, kt, t] = xs[t, kt*128+kp]
        xt = np.ascontiguousarray(
            xs.T.reshape(KT, 128, T_C).transpose(1, 0, 2)
        ).astype(np_cdt)
        in_maps.append({"xt": xt, "wt": wt_prep, "bias": bias_prep})

    nc = _get_nc(compute_dt)
    res = run_bass_kernel_spmd(nc, in_maps, list(range(N_CORES)), trace=_trace)
    kernel.last_results = res

    y = np.empty((TOKENS, D_OUT), np.float32)
    for c in range(N_CORES):
        y[c * T_C : (c + 1) * T_C] = res.results[c]["out"].T
    return y
